# revision 15
# baseline (speedup 1.0000x reference)
"""Trainium2 Bass kernel for nn_CondenseSFR (BN+ReLU+shuffle+grouped1x1conv+reindex).

Algebra: out = einsum('nchw,cd->ndhw', conv(shuffle(relu(bn(x)))), index).
Everything except the ReLU is linear in the channel dimension, and the BN
scale inv = gamma*rsqrt(var+eps) is strictly positive, so
    relu(inv*x + b) = inv * relu(x + b/inv)
and the shuffle + grouped conv + reindex fold into a single dense 512x512
channel matrix applied after the ReLU:
    out[n,d,s] = sum_c B[d,c] * relu(x[n,c,s] + bprime[c])
with B = (index^T @ A) * inv[None,:],  A the shuffle-permuted block-diagonal
conv weight, bprime = (beta - mean*inv)/inv.

Device work per core (4 of 32 images, data-parallel over 8 cores):
  - DMA x image [512, 1024+1] as 4 partition-tiles [128, 1025]; the extra
    column carries the per-channel ReLU bias so each ReLU has exactly one
    DMA dependency (2-wait instructions cost a standalone event-semaphore)
  - ScalarE: relu(x + bias_c) per channel tile, output rounded to fp32r
  - TensorE: per (d-tile): 2 spatial halves x 4 c-tile accumulation steps
    of (128x128) @ (128x512) fp32r matmuls into PSUM
  - VectorE: PSUM -> SBUF copies; per-d-tile 512KB stores drain the output
    stream early instead of one trailing 2MB store
"""

import numpy as np

import concourse.bacc as bacc
import concourse.mybir as mybir
from concourse.tile import TileContext
from concourse.bass_utils import run_bass_kernel_spmd

EPS = 1e-5
GROUPS = 4
N, C, H, W = 32, 512, 32, 32
HW = H * W                 # 1024
HWB = HW + 1               # +1 bias column per channel tile
NCORES = 8
NPER = N // NCORES         # 4 images per core
CT = C // 128              # 4 channel tiles
F32 = mybir.dt.float32
F32R = mybir.dt.float32r

_NC_CACHE = None


def _build_nc():
    """Build the (SPMD, per-core) Bass program. Same program on all 8 cores."""
    nc = bacc.Bacc(None, enable_partition_id=False)

    x_d = nc.dram_tensor("x", [NPER, CT, 128, HWB], F32, kind="ExternalInput")
    w_d = nc.dram_tensor("w", [128, CT * CT * 128], F32R, kind="ExternalInput")
    o_d = nc.dram_tensor("o", [NPER, CT, 128, HW], F32, kind="ExternalOutput")

    with TileContext(nc) as tc:
        with (
            tc.tile_pool(name="const", bufs=1) as const,
            tc.tile_pool(name="xin", bufs=4) as xin,
            tc.tile_pool(name="act", bufs=3) as actp,
            tc.tile_pool(name="pp", bufs=8, space="PSUM") as pp,
            tc.tile_pool(name="outp", bufs=2) as outp,
        ):
            # Weight DMA first on the Scalar HWDGE ring: Sync's ring stays
            # free for the first x chunk, which gates the first ReLU.
            wt = const.tile([128, CT * CT * 128], F32R)
            nc.scalar.dma_start(wt[:], w_d[:])

            # Pre-issue every input chunk across BOTH HWDGE rings: a single
            # DMA queue sustains only ~250 GB/s, two active queues reach the
            # ~410 GB/s HBM cap. bufs=NPER so no trigger ever waits on a slot.
            xts = []
            for n in range(NPER):
                xt = xin.tile([128, CT * HWB], F32, name=f"xt{n}", tag="xt")
                xts.append(xt)
                for ct in range(CT):
                    eng = nc.sync if ct % 2 == 0 else nc.scalar
                    eng.dma_start(xt[:, ct * HWB:(ct + 1) * HWB], x_d[n, ct])

            for n in range(NPER):
                xt = xts[n]
                ut = actp.tile([128, CT * HW], F32R)
                # 8 PSUM banks accumulate ct-major, so matmuls start as soon
                # as the first channel tile lands instead of after the last.
                pss = [
                    pp.tile([128, 512], F32, name=f"ps_{n}_{j}", tag=f"ps{j}", bufs=1)
                    for j in range(2 * CT)
                ]
                for ct in range(CT):
                    nc.scalar.activation(
                        ut[:, ct * HW:(ct + 1) * HW],
                        xt[:, ct * HWB:ct * HWB + HW],
                        mybir.ActivationFunctionType.Relu,
                        bias=xt[:, ct * HWB + HW:(ct + 1) * HWB],
                    )
                    for dt_ in range(CT):
                        for half in range(2):
                            wcol = (ct * CT + dt_) * 128
                            ucol = ct * HW + half * 512
                            nc.tensor.matmul(
                                pss[dt_ * 2 + half][:],
                                wt[:, wcol:wcol + 128],
                                ut[:, ucol:ucol + 512],
                                start=(ct == 0),
                                stop=(ct == CT - 1),
                            )

                last = n == NPER - 1
                ot = outp.tile([128, CT * HW], F32)
                for dt_ in range(CT):
                    for half in range(2):
                        ocol = dt_ * HW + half * 512
                        ps = pss[dt_ * 2 + half]
                        # split the last image's drain across DVE and ACT;
                        # earlier images fit easily on DVE alone
                        if half == 1 and last:
                            nc.scalar.copy(ot[:, ocol:ocol + 512], ps[:])
                        else:
                            nc.vector.tensor_copy(ot[:, ocol:ocol + 512], ps[:])
                    # Early stores ride the GpSimd SWDGE queue (3rd DMA
                    # queue, doesn't steal HWDGE input bandwidth and a
                    # waiting store can't block loads/relus). The last
                    # image's stores use the HWDGE rings, which are idle
                    # once the input stream finishes.
                    if last:
                        seng = nc.sync if dt_ % 2 == 0 else nc.scalar
                        seng.dma_start(o_d[n, dt_], ot[:, dt_ * HW:(dt_ + 1) * HW])
                    else:
                        nc.gpsimd.dma_start(
                            o_d[n, dt_], ot[:, dt_ * HW:(dt_ + 1) * HW]
                        )

    nc.finalize()
    return nc


def _prep_inputs(x, gamma, beta, running_mean, running_var, weight, index):
    """Fold BN/shuffle/conv/index into (per-core x shards, weight matrix)."""
    f64 = np.float64
    gamma = gamma.astype(f64)
    beta = beta.astype(f64)
    mean = running_mean.astype(f64)
    var = running_var.astype(f64)
    Wc = weight.reshape(C, C // GROUPS).astype(f64)   # (Cout, Cin_per_group)
    idx = index.astype(f64)

    inv = gamma / np.sqrt(var + EPS)                  # > 0
    beta_term = beta - mean * inv
    inv_safe = np.where(inv != 0.0, inv, 1.0)
    bprime = np.where(inv != 0.0, beta_term / inv_safe, 0.0)

    # A[o, c]: conv-after-shuffle as one 512x512 matrix.
    # shuffled channel g*128 + i comes from original channel i*GROUPS + g.
    A = np.zeros((C, C), dtype=f64)
    o = np.arange(C)
    i = np.arange(C // GROUPS)
    src = i[None, :] * GROUPS + (o[:, None] // (C // GROUPS))  # (512, 128)
    A[o[:, None], src] = Wc

    # out[d] = sum_c B[d,c] relu(x_c + bprime_c);  B = (idx^T @ A) * inv
    # Stationary operand is B^T[c, d] = (A^T @ idx) * inv[:, None]
    BT = (A.T @ idx) * inv[:, None]                   # (c, d)

    w_host = np.ascontiguousarray(
        BT.reshape(CT, 128, CT, 128).transpose(1, 0, 2, 3).reshape(128, CT * CT * 128)
    ).astype(np.float32)

    # x shards with the bias appended as column HW of each [128, HW] block
    xr = x.reshape(N, CT, 128, HW)
    bias_col = np.broadcast_to(
        bprime.astype(np.float32).reshape(CT, 128, 1), (N, CT, 128, 1)
    )
    xaug = np.concatenate([xr, bias_col], axis=3)      # (N, CT, 128, HWB)
    xaug = np.ascontiguousarray(
        xaug.reshape(NCORES, NPER, CT, 128, HWB), dtype=np.float32
    )
    return [{"x": xaug[k], "w": w_host} for k in range(NCORES)]


def _run(inputs, trace=False):
    global _NC_CACHE
    if _NC_CACHE is None:
        _NC_CACHE = _build_nc()
    in_maps = _prep_inputs(**inputs)
    res = run_bass_kernel_spmd(_NC_CACHE, in_maps, list(range(NCORES)), trace=trace)
    out = np.concatenate([res.results[k]["o"] for k in range(NCORES)], axis=0)
    out = out.reshape(N, C, H, W).astype(np.float32)
    return out, res


def kernel(**inputs):
    out, _ = _run(inputs, trace=False)
    return out


# revision 16
# speedup vs baseline: 1.0186x; 1.0186x over previous
"""Trainium2 Bass kernel for nn_CondenseSFR (BN+ReLU+shuffle+grouped1x1conv+reindex).

Algebra: out = einsum('nchw,cd->ndhw', conv(shuffle(relu(bn(x)))), index).
Everything except the ReLU is linear in the channel dimension, and the BN
scale inv = gamma*rsqrt(var+eps) is strictly positive, so
    relu(inv*x + b) = inv * relu(x + b/inv)
and the shuffle + grouped conv + reindex fold into a single dense 512x512
channel matrix applied after the ReLU:
    out[n,d,s] = sum_c B[d,c] * relu(x[n,c,s] + bprime[c])
with B = (index^T @ A) * inv[None,:],  A the shuffle-permuted block-diagonal
conv weight, bprime = (beta - mean*inv)/inv.

Device work per core (4 of 32 images, data-parallel over 8 cores):
  - DMA x image [512, 1024+1] as 4 partition-tiles [128, 1025]; the extra
    column carries the per-channel ReLU bias so each ReLU has exactly one
    DMA dependency (2-wait instructions cost a standalone event-semaphore)
  - ScalarE: relu(x + bias_c) per channel tile, output rounded to fp32r
  - TensorE: per (d-tile): 2 spatial halves x 4 c-tile accumulation steps
    of (128x128) @ (128x512) fp32r matmuls into PSUM
  - VectorE: PSUM -> SBUF copies; per-d-tile 512KB stores drain the output
    stream early instead of one trailing 2MB store
"""

import numpy as np

import concourse.bacc as bacc
import concourse.mybir as mybir
from concourse.tile import TileContext
from concourse.bass_utils import run_bass_kernel_spmd

EPS = 1e-5
GROUPS = 4
N, C, H, W = 32, 512, 32, 32
HW = H * W                 # 1024
HWB = HW + 1               # +1 bias column per channel tile
NCORES = 8
NPER = N // NCORES         # 4 images per core
CT = C // 128              # 4 channel tiles
F32 = mybir.dt.float32
F32R = mybir.dt.float32r

_NC_CACHE = None


def _build_nc():
    """Build the (SPMD, per-core) Bass program. Same program on all 8 cores."""
    nc = bacc.Bacc(None, enable_partition_id=False)

    x_d = nc.dram_tensor("x", [NPER, CT, 128, HWB], F32, kind="ExternalInput")
    w_d = nc.dram_tensor("w", [128, CT * CT * 128], F32R, kind="ExternalInput")
    o_d = nc.dram_tensor("o", [NPER, CT, 128, HW], F32, kind="ExternalOutput")

    with TileContext(nc) as tc:
        with (
            tc.tile_pool(name="const", bufs=1) as const,
            tc.tile_pool(name="xin", bufs=4) as xin,
            tc.tile_pool(name="act", bufs=3) as actp,
            tc.tile_pool(name="pp", bufs=8, space="PSUM") as pp,
            tc.tile_pool(name="outp", bufs=2) as outp,
        ):
            # Weight DMA first on the Scalar HWDGE ring: Sync's ring stays
            # free for the first x chunk, which gates the first ReLU.
            wt = const.tile([128, CT * CT * 128], F32R)
            nc.scalar.dma_start(wt[:], w_d[:])

            # Pre-issue every input chunk across BOTH HWDGE rings: a single
            # DMA queue sustains only ~250 GB/s, two active queues reach the
            # ~410 GB/s HBM cap. bufs=NPER so no trigger ever waits on a slot.
            xts = []
            for n in range(NPER):
                xt = xin.tile([128, CT * HWB], F32, name=f"xt{n}", tag="xt")
                xts.append(xt)
                for ct in range(CT):
                    eng = nc.sync if ct % 2 == 0 else nc.scalar
                    eng.dma_start(xt[:, ct * HWB:(ct + 1) * HWB], x_d[n, ct])

            for n in range(NPER):
                xt = xts[n]
                ut = actp.tile([128, CT * HW], F32R)
                # 8 PSUM banks accumulate ct-major, so matmuls start as soon
                # as the first channel tile lands instead of after the last.
                pss = [
                    pp.tile([128, 512], F32, name=f"ps_{n}_{j}", tag=f"ps{j}", bufs=1)
                    for j in range(2 * CT)
                ]
                for ct in range(CT):
                    # relu(x + b) on DVE (fp32 tensor_scalar runs 2x there,
                    # and keeps ScalarE free to feed its HWDGE DMA ring)
                    nc.vector.tensor_scalar(
                        ut[:, ct * HW:(ct + 1) * HW],
                        xt[:, ct * HWB:ct * HWB + HW],
                        xt[:, ct * HWB + HW:(ct + 1) * HWB],
                        0.0,
                        mybir.AluOpType.add,
                        mybir.AluOpType.max,
                    )
                    for dt_ in range(CT):
                        for half in range(2):
                            wcol = (ct * CT + dt_) * 128
                            ucol = ct * HW + half * 512
                            nc.tensor.matmul(
                                pss[dt_ * 2 + half][:],
                                wt[:, wcol:wcol + 128],
                                ut[:, ucol:ucol + 512],
                                start=(ct == 0),
                                stop=(ct == CT - 1),
                            )

                last = n == NPER - 1
                ot = outp.tile([128, CT * HW], F32)
                for dt_ in range(CT):
                    for half in range(2):
                        ocol = dt_ * HW + half * 512
                        ps = pss[dt_ * 2 + half]
                        # split the last image's drain across DVE and ACT;
                        # earlier images fit easily on DVE alone
                        if half == 1 and last:
                            nc.scalar.copy(ot[:, ocol:ocol + 512], ps[:])
                        else:
                            nc.vector.tensor_copy(ot[:, ocol:ocol + 512], ps[:])
                    # Early stores ride the GpSimd SWDGE queue (3rd DMA
                    # queue, doesn't steal HWDGE input bandwidth and a
                    # waiting store can't block loads/relus). The last
                    # image's stores use the HWDGE rings, which are idle
                    # once the input stream finishes.
                    if last:
                        seng = nc.sync if dt_ % 2 == 0 else nc.scalar
                        seng.dma_start(o_d[n, dt_], ot[:, dt_ * HW:(dt_ + 1) * HW])
                    else:
                        nc.gpsimd.dma_start(
                            o_d[n, dt_], ot[:, dt_ * HW:(dt_ + 1) * HW]
                        )

    nc.finalize()
    return nc


def _prep_inputs(x, gamma, beta, running_mean, running_var, weight, index):
    """Fold BN/shuffle/conv/index into (per-core x shards, weight matrix)."""
    f64 = np.float64
    gamma = gamma.astype(f64)
    beta = beta.astype(f64)
    mean = running_mean.astype(f64)
    var = running_var.astype(f64)
    Wc = weight.reshape(C, C // GROUPS).astype(f64)   # (Cout, Cin_per_group)
    idx = index.astype(f64)

    inv = gamma / np.sqrt(var + EPS)                  # > 0
    beta_term = beta - mean * inv
    inv_safe = np.where(inv != 0.0, inv, 1.0)
    bprime = np.where(inv != 0.0, beta_term / inv_safe, 0.0)

    # A[o, c]: conv-after-shuffle as one 512x512 matrix.
    # shuffled channel g*128 + i comes from original channel i*GROUPS + g.
    A = np.zeros((C, C), dtype=f64)
    o = np.arange(C)
    i = np.arange(C // GROUPS)
    src = i[None, :] * GROUPS + (o[:, None] // (C // GROUPS))  # (512, 128)
    A[o[:, None], src] = Wc

    # out[d] = sum_c B[d,c] relu(x_c + bprime_c);  B = (idx^T @ A) * inv
    # Stationary operand is B^T[c, d] = (A^T @ idx) * inv[:, None]
    BT = (A.T @ idx) * inv[:, None]                   # (c, d)

    w_host = np.ascontiguousarray(
        BT.reshape(CT, 128, CT, 128).transpose(1, 0, 2, 3).reshape(128, CT * CT * 128)
    ).astype(np.float32)

    # x shards with the bias appended as column HW of each [128, HW] block
    xr = x.reshape(N, CT, 128, HW)
    bias_col = np.broadcast_to(
        bprime.astype(np.float32).reshape(CT, 128, 1), (N, CT, 128, 1)
    )
    xaug = np.concatenate([xr, bias_col], axis=3)      # (N, CT, 128, HWB)
    xaug = np.ascontiguousarray(
        xaug.reshape(NCORES, NPER, CT, 128, HWB), dtype=np.float32
    )
    return [{"x": xaug[k], "w": w_host} for k in range(NCORES)]


def _run(inputs, trace=False):
    global _NC_CACHE
    if _NC_CACHE is None:
        _NC_CACHE = _build_nc()
    in_maps = _prep_inputs(**inputs)
    res = run_bass_kernel_spmd(_NC_CACHE, in_maps, list(range(NCORES)), trace=trace)
    out = np.concatenate([res.results[k]["o"] for k in range(NCORES)], axis=0)
    out = out.reshape(N, C, H, W).astype(np.float32)
    return out, res


def kernel(**inputs):
    out, _ = _run(inputs, trace=False)
    return out
